# revision 11
# baseline (speedup 1.0000x reference)
"""Trainium2 Bass kernel for nn_FCLSTM: embedding -> custom LSTM-ish recurrence -> select -> linear -> log_softmax.

Batch-sharded: each of 8 cores owns 8 batch rows. Per core:
  phase B: gather embeddings for its 4096 tokens, inp = relu(e @ Wi.T + bi) -> DRAM
  phase C: 512-step recurrence on its 8 rows (full H=1024), h stored per step to ring
  phase D: select h at lengths-1, tiny AllGather [8,1024]->[64,1024], replicated final linear+log_softmax

Self-contained: hardcodes shapes. kernel(**inputs) takes full numpy inputs, returns [64, 2] fp32.
"""
import os
import numpy as np

import concourse.bacc as bacc
import concourse.bass as bass
import concourse.mybir as mybir
from concourse import library_config  # noqa: F401
from concourse.tile import TileContext
from concourse.masks import make_identity
from concourse.bass_utils import run_bass_kernel_spmd

VOCAB, EMBED, HIDDEN, NCLS = 32000, 512, 1024, 2
B, S = 64, 512
NCORES = 8
BC = B // NCORES               # 8 batch rows per core
NEC = EMBED // 128             # 4 embed contraction chunks
NKC = HIDDEN // 128            # 8 hidden contraction chunks
F16 = mybir.dt.float16
F32 = mybir.dt.float32
I32 = mybir.dt.int32
AF = mybir.ActivationFunctionType

_CACHE = {}


def _build(steps=S):
    nc = bacc.Bacc("TRN2", target_bir_lowering=False, debug=False, num_devices=NCORES)
    ntok = BC * steps
    ntt = ntok // 128              # token tiles (32 for full S)

    # ---------- inputs ----------
    emb = nc.dram_tensor("emb", [VOCAB, EMBED], F16, kind="ExternalInput")
    wi = nc.dram_tensor("wi", [EMBED, HIDDEN], F16, kind="ExternalInput")
    bi = nc.dram_tensor("bi", [1, HIDDEN], F16, kind="ExternalInput")
    wf = nc.dram_tensor("wf", [HIDDEN, HIDDEN], F16, kind="ExternalInput")
    wh = nc.dram_tensor("wh", [HIDDEN, HIDDEN], F16, kind="ExternalInput")
    bf_r = nc.dram_tensor("bf_r", [1, HIDDEN], F16, kind="ExternalInput")
    bh_r = nc.dram_tensor("bh_r", [1, HIDDEN], F16, kind="ExternalInput")
    wo = nc.dram_tensor("wo", [HIDDEN, HIDDEN], F16, kind="ExternalInput")
    bo_r = nc.dram_tensor("bo_r", [1, HIDDEN], F16, kind="ExternalInput")
    wlin = nc.dram_tensor("wlin", [HIDDEN, NCLS], F16, kind="ExternalInput")
    idxg = nc.dram_tensor("idxg", [128, ntt], I32, kind="ExternalInput")
    selidx = nc.dram_tensor("selidx", [128, 1], I32, kind="ExternalInput")
    out_ext = nc.dram_tensor("out", [B, NCLS], F32, kind="ExternalOutput")

    with TileContext(nc) as tc:
        with (
            tc.tile_pool(name="dram", bufs=1, space="DRAM") as dram,
            tc.tile_pool(name="const", bufs=1) as cst,
            tc.tile_pool(name="w", bufs=1) as wpool,
            tc.tile_pool(name="ph", bufs=3) as ph,
            tc.tile_pool(name="ppsum", bufs=2, space="PSUM") as ppsum,
            tc.tile_pool(name="gpsum", bufs=2, space="PSUM") as gpsum,
            tc.tile_pool(name="tpsum", bufs=2, space="PSUM") as tpsum,
            tc.tile_pool(name="rec", bufs=3) as rec,
            tc.tile_pool(name="inp", bufs=6) as inpp,
        ):
            # ---------- DRAM scratch ----------
            inp_dram = dram.tile([ntok, HIDDEN], F16)
            ring = dram.tile([ntok, HIDDEN], F16)
            agin = dram.tile([BC, HIDDEN], F16)
            gath = dram.tile([B, HIDDEN], F16, addr_space="Shared")

            # ---------- constants / weights to SBUF ----------
            ones = cst.tile([1, 128], F16, tag="ones")
            nc.vector.memset(ones[:], 1.0)
            ident128 = cst.tile([128, 128], F16, tag="id128")
            make_identity(nc, ident128[:])
            ident8 = cst.tile([8, 8], F16, tag="id8")
            make_identity(nc, ident8[:])
            ident8h = cst.tile([72, 8], F16, tag="id8h")
            nc.vector.tensor_copy(out=ident8h[64:72, :], in_=ident8[:])
            ident64 = cst.tile([64, 64], F16, tag="id64")
            make_identity(nc, ident64[:])

            wi_sb = cst.tile([128, NEC * HIDDEN], F16, tag="wi")
            for e in range(NEC):
                nc.sync.dma_start(out=wi_sb[:, e * HIDDEN:(e + 1) * HIDDEN],
                                  in_=wi[e * 128:(e + 1) * 128, :])
            bi_sb = cst.tile([1, HIDDEN], F16, tag="bi")
            nc.sync.dma_start(out=bi_sb[:], in_=bi[:])
            bf_sb = cst.tile([1, HIDDEN], F16, tag="bf")
            nc.sync.dma_start(out=bf_sb[:], in_=bf_r[:])
            bh_sb = cst.tile([1, HIDDEN], F16, tag="bh")
            nc.sync.dma_start(out=bh_sb[:], in_=bh_r[:])
            bo_sb = cst.tile([1, HIDDEN], F16, tag="bo")
            nc.sync.dma_start(out=bo_sb[:], in_=bo_r[:])

            wf_sb = wpool.tile([128, NKC * HIDDEN], F16, tag="wf")
            wh_sb = wpool.tile([128, NKC * HIDDEN], F16, tag="wh")
            for k in range(NKC):
                nc.sync.dma_start(out=wf_sb[:, k * HIDDEN:(k + 1) * HIDDEN],
                                  in_=wf[k * 128:(k + 1) * 128, :])
                nc.sync.dma_start(out=wh_sb[:, k * HIDDEN:(k + 1) * HIDDEN],
                                  in_=wh[k * 128:(k + 1) * 128, :])

            # ---------- phase B: inp = relu(e @ Wi.T + bi) for this core's tokens ----------
            idx_sb = cst.tile([128, ntt], I32, tag="idx")
            nc.sync.dma_start(out=idx_sb[:], in_=idxg[:, :])
            for i in range(ntt):
                gt = ph.tile([128, EMBED], F16, tag="gt")
                nc.gpsimd.indirect_dma_start(
                    out=gt[:], out_offset=None,
                    in_=emb[:, :],
                    in_offset=bass.IndirectOffsetOnAxis(ap=idx_sb[:, i:i + 1], axis=0))
                pte = tpsum.tile([128, 512], F16, tag="pt")
                for e in range(NEC):
                    nc.tensor.transpose(out=pte[:, e * 128:(e + 1) * 128],
                                        in_=gt[:, e * 128:(e + 1) * 128],
                                        identity=ident128[:])
                eT = ph.tile([128, EMBED], F16, tag="eT")
                nc.vector.tensor_copy(out=eT[:], in_=pte[:])
                inp_sb = ph.tile([128, HIDDEN], F16, tag="inp_sb")
                for n in range(2):
                    ns = slice(n * 512, (n + 1) * 512)
                    pu = ppsum.tile([128, 512], F32, tag="pu")
                    for e in range(NEC):
                        nc.tensor.matmul(out=pu[:], lhsT=eT[:, e * 128:(e + 1) * 128],
                                         rhs=wi_sb[:, e * HIDDEN + n * 512:e * HIDDEN + (n + 1) * 512],
                                         start=(e == 0), stop=False)
                    nc.tensor.matmul(out=pu[:], lhsT=ones[:, 0:128], rhs=bi_sb[:, ns],
                                     start=False, stop=True)
                    nc.scalar.activation(inp_sb[:, ns], pu[:], AF.Relu)
                nc.sync.dma_start(out=inp_dram[i * 128:(i + 1) * 128, :], in_=inp_sb[:])

            # ---------- phase C: recurrence over this core's 8 rows ----------
            # hT chunks: hT[:, 8k:8k+8] = h[:, 128k:128(k+1)].T  ([128, 8] per chunk)
            # 4-way col-tiled PSUM quarters (pg is [128, 512]):
            #   zf-n0 @ rows 0:8, zh-n0 @ 32:40, zf-n1 @ 64:72, zh-n1 @ 96:104
            # tanh(z) = 2*sigmoid(2z) - 1, so ONE sigmoid act covers all quarters
            # via a per-partition scale vector (1 on zf rows, 2 on zh rows).
            scl = cst.tile([128, 1], F32, tag="scl")
            nc.vector.memset(scl[:], 1.0)
            nc.vector.memset(scl[32:40, :], 2.0)
            nc.vector.memset(scl[96:104, :], 2.0)

            # State tiles live at two partition bases so TensorTensor ops stay
            # base-aligned: n0 data (H cols 0:512) at rows 0:8, n1 data
            # (H cols 512:1024) at rows 64:72.
            hT0 = rec.tile([128, NKC * BC], F16, tag="hT")
            nc.vector.memset(hT0[:], 0.0)
            hnw_prev = None
            KORD = [4, 5, 0, 1, 6, 7, 2, 3]  # consumption order matches chunk readiness

            inp_tiles = {}

            def load_inp(t):
                if t >= steps:
                    return
                it = inpp.tile([72, 512], F16, tag="inp")
                nc.sync.dma_start(out=it[0:BC, :], in_=inp_dram[t * BC:(t + 1) * BC, 0:512])
                nc.sync.dma_start(out=it[64:64 + BC, :], in_=inp_dram[t * BC:(t + 1) * BC, 512:1024])
                inp_tiles[t] = it

            load_inp(0)
            load_inp(1)

            for t in range(steps):
                load_inp(t + 2)
                inp = inp_tiles.pop(t)

                pgA = gpsum.tile([128, 256], F32, tag="pgA", name="pgA")
                pgB = gpsum.tile([128, 256], F32, tag="pgB", name="pgB")
                pgs = [pgA, pgB]
                if t > 0:
                    hTc = rec.tile([128, NKC * BC], F16, tag="hT")
                    pt = tpsum.tile([128, 512], F16, tag="pt")

                    def tr2(k0):
                        # transpose prev-step chunks k0, k0+1 and land them in hTc
                        for kk in (k0, k0 + 1):
                            if kk < 4:
                                src = hnw_prev[0:BC, kk * 128:(kk + 1) * 128]
                                idn = ident8[:]
                            else:
                                src = hnw_prev[64:64 + BC, (kk - 4) * 128:(kk - 3) * 128]
                                idn = ident8h[64:72, :]
                            nc.tensor.transpose(out=pt[:, kk * BC:(kk + 1) * BC],
                                                in_=src, identity=idn)
                        nc.scalar.copy(out=hTc[:, k0 * BC:(k0 + 2) * BC],
                                       in_=pt[:, k0 * BC:(k0 + 2) * BC])
                else:
                    hTc = hT0

                    def tr2(k0):
                        pass

                def gates4(k, p, stop):
                    # 4 concurrent MMs: (f,n0) (h,n0) (f,n1) (h,n1) on col groups 0..3
                    lhs = hTc[:, k * BC:(k + 1) * BC]
                    for q in range(4):
                        nn, wsb = q // 2, (wf_sb if q % 2 == 0 else wh_sb)
                        w0 = k * HIDDEN + nn * 512 + p * 256
                        nc.tensor.matmul(out=pgs[p][32 * q:32 * q + BC, :], lhsT=lhs,
                                         rhs=wsb[:, w0:w0 + 256],
                                         start=False, stop=stop, tile_position=(0, 32 * q))

                def bias4(p):
                    for q in range(4):
                        nn, bsb = q // 2, (bf_sb if q % 2 == 0 else bh_sb)
                        b0 = nn * 512 + p * 256
                        nc.tensor.matmul(out=pgs[p][32 * q:32 * q + BC, :], lhsT=ones[:, 0:BC],
                                         rhs=bsb[:, b0:b0 + 256],
                                         start=True, stop=False, tile_position=(0, 32 * q))

                sg = rec.tile([128, 512], F16, tag="sg")
                ut = rec.tile([72, 512], F16, tag="ut")
                hnw = rec.tile([72, 512], F16, tag="hnw")

                def tail(p):
                    # one act covers all 4 quarters (tanh via 2*sigmoid(2z)-1);
                    # n1 chain on DVE, n0 chain on GpSimd, in parallel
                    pc = slice(p * 256, (p + 1) * 256)
                    nc.scalar.activation(sg[0:104, pc], pgs[p][0:104, :], AF.Sigmoid,
                                         scale=scl[0:104, :])
                    for nn, eng in ((1, nc.vector), (0, nc.gpsimd)):
                        b = 64 * nn
                        eng.tensor_scalar(out=ut[b:b + BC, pc], in0=sg[b + 32:b + 40, pc],
                                          scalar1=2.0, scalar2=1.0,
                                          op0=mybir.AluOpType.mult,
                                          op1=mybir.AluOpType.subtract)
                        eng.tensor_mul(out=hnw[b:b + BC, pc], in0=ut[b:b + BC, pc],
                                       in1=inp[b:b + BC, pc])
                        eng.tensor_add(out=hnw[b:b + BC, pc], in0=hnw[b:b + BC, pc],
                                       in1=sg[b:b + BC, pc])

                # PE program order: prev-step transposes interleave as fillers
                tr2(4)
                bias4(0)
                tr2(0)
                gates4(4, 0, False)
                gates4(5, 0, False)
                gates4(0, 0, False)
                gates4(1, 0, False)
                tr2(6)
                gates4(6, 0, False)
                gates4(7, 0, False)
                tr2(2)
                gates4(2, 0, False)
                gates4(3, 0, True)
                tail(0)
                bias4(1)
                for i, k in enumerate(KORD):
                    gates4(k, 1, i == 7)
                tail(1)

                nc.sync.dma_start(out=ring[t * BC:(t + 1) * BC, 0:512], in_=hnw[0:BC, :])
                nc.sync.dma_start(out=ring[t * BC:(t + 1) * BC, 512:1024],
                                  in_=hnw[64:64 + BC, :])
                hnw_prev = hnw

            # ---------- phase D: select + AllGather + linear + log_softmax ----------
            six = cst.tile([128, 1], I32, tag="six")
            nc.sync.dma_start(out=six[:], in_=selidx[:])
            hsel = cst.tile([128, HIDDEN], F16, tag="hsel")
            nc.gpsimd.indirect_dma_start(
                out=hsel[:], out_offset=None,
                in_=ring[:, :],
                in_offset=bass.IndirectOffsetOnAxis(ap=six[:, :1], axis=0))
            nc.sync.dma_start(out=agin[:, :], in_=hsel[0:BC, :])
            nc.gpsimd.collective_compute(
                "AllGather", mybir.AluOpType.bypass,
                replica_groups=[list(range(NCORES))],
                ins=[agin.opt()], outs=[gath.opt()])
            h64 = cst.tile([64, HIDDEN], F16, tag="h64")
            nc.sync.dma_start(out=h64[:], in_=gath[:, :])

            pt2 = tpsum.tile([128, 512], F16, tag="pt")
            for k in range(NKC):
                nc.tensor.transpose(out=pt2[:, k * 64:(k + 1) * 64],
                                    in_=h64[:, k * 128:(k + 1) * 128],
                                    identity=ident64[:])
            hT64 = cst.tile([128, NKC * 64], F16, tag="hT64")
            nc.vector.tensor_copy(out=hT64[:], in_=pt2[:])

            wo_sb = wpool.tile([128, NKC * HIDDEN], F16, tag="wo")
            for k in range(NKC):
                nc.sync.dma_start(out=wo_sb[:, k * HIDDEN:(k + 1) * HIDDEN],
                                  in_=wo[k * 128:(k + 1) * 128, :])
            lin = cst.tile([64, HIDDEN], F16, tag="lin")
            for n in range(2):
                ns = slice(n * 512, (n + 1) * 512)
                pl = ppsum.tile([128, 512], F32, tag="pu")
                for k in range(NKC):
                    nc.tensor.matmul(out=pl[0:64, :], lhsT=hT64[:, k * 64:(k + 1) * 64],
                                     rhs=wo_sb[:, k * HIDDEN + n * 512:k * HIDDEN + (n + 1) * 512],
                                     start=(k == 0), stop=False)
                nc.tensor.matmul(out=pl[0:64, :], lhsT=ones[:, 0:64], rhs=bo_sb[:, ns],
                                 start=False, stop=True)
                nc.vector.tensor_copy(out=lin[:, ns], in_=pl[0:64, :])
            pt3 = tpsum.tile([128, 512], F16, tag="pt")
            for k in range(NKC):
                nc.tensor.transpose(out=pt3[:, k * 64:(k + 1) * 64],
                                    in_=lin[:, k * 128:(k + 1) * 128],
                                    identity=ident64[:])
            linT = cst.tile([128, NKC * 64], F16, tag="linT")
            nc.vector.tensor_copy(out=linT[:], in_=pt3[:])
            wl_sb = cst.tile([128, NKC * NCLS], F16, tag="wl")
            for k in range(NKC):
                nc.sync.dma_start(out=wl_sb[:, k * NCLS:(k + 1) * NCLS],
                                  in_=wlin[k * 128:(k + 1) * 128, :])
            pz = ppsum.tile([128, 512], F32, tag="pu")
            for k in range(NKC):
                nc.tensor.matmul(out=pz[0:64, 0:NCLS], lhsT=linT[:, k * 64:(k + 1) * 64],
                                 rhs=wl_sb[:, k * NCLS:(k + 1) * NCLS],
                                 start=(k == 0), stop=(k == NKC - 1))
            m = cst.tile([64, 1], F32, tag="m")
            nc.vector.tensor_reduce(out=m[:], in_=pz[0:64, 0:NCLS], axis=mybir.AxisListType.X,
                                    op=mybir.AluOpType.max)
            xm = cst.tile([64, NCLS], F32, tag="xm")
            nc.vector.tensor_scalar(out=xm[:], in0=pz[0:64, 0:NCLS], scalar1=m[:], scalar2=None,
                                    op0=mybir.AluOpType.subtract)
            esum = cst.tile([64, 1], F32, tag="esum")
            ex = cst.tile([64, NCLS], F32, tag="ex")
            nc.scalar.activation(ex[:], xm[:], AF.Exp, accum_out=esum[:])
            lns = cst.tile([64, 1], F32, tag="lns")
            nc.scalar.activation(lns[:], esum[:], AF.Ln)
            res = cst.tile([64, NCLS], F32, tag="res")
            nc.vector.tensor_scalar(out=res[:], in0=xm[:], scalar1=lns[:], scalar2=None,
                                    op0=mybir.AluOpType.subtract)
            nc.sync.dma_start(out=out_ext[:, :], in_=res[:])

    nc.compile()
    return nc


def _prep(x, lengths, emb, W_i, b_i, W_f, b_f, W_h, b_h, W_o, b_o, W_lin, b_lin,
          steps=S):
    f16 = np.float16
    emb16 = np.ascontiguousarray(emb.astype(f16))
    wiT = np.ascontiguousarray(W_i.T.astype(f16))
    wfT = np.ascontiguousarray(W_f.T.astype(f16))
    whT = np.ascontiguousarray(W_h.T.astype(f16))
    woT = np.ascontiguousarray(W_o.T.astype(f16))
    wlT = np.ascontiguousarray(W_lin.T.astype(f16))
    maps = []
    for c in range(NCORES):
        rows = slice(c * BC, (c + 1) * BC)
        tok = np.ascontiguousarray(x[rows, :steps].T).reshape(-1)  # t-major [steps*BC]
        ntt = (steps * BC) // 128
        idxg = np.ascontiguousarray(tok.reshape(ntt, 128).T).astype(np.int32)
        ln = np.minimum(lengths[rows].astype(np.int64), steps)
        sel = ((ln - 1) * BC + np.arange(BC)).astype(np.int32)
        selpad = np.zeros((128, 1), np.int32)
        selpad[:BC, 0] = sel
        maps.append({
            "emb": emb16,
            "wi": wiT,
            "bi": b_i[None, :].astype(f16),
            "wf": wfT,
            "wh": whT,
            "bf_r": b_f[None, :].astype(f16),
            "bh_r": b_h[None, :].astype(f16),
            "wo": woT,
            "bo_r": b_o[None, :].astype(f16),
            "wlin": wlT,
            "idxg": idxg,
            "selidx": selpad,
        })
    return maps


def _run(inputs, steps=S, trace=False):
    key = steps
    if key not in _CACHE:
        _CACHE[key] = _build(steps)
    nc = _CACHE[key]
    maps = _prep(**inputs, steps=steps)
    res = run_bass_kernel_spmd(nc, maps, core_ids=list(range(NCORES)), trace=trace)
    return res


def kernel(**inputs) -> np.ndarray:
    res = _run(inputs, steps=S, trace=False)
    return res.results[0]["out"]


if __name__ == "__main__":
    steps = int(os.environ.get("KSTEPS", "16"))
    rng = np.random.default_rng(0)
    x = rng.integers(0, VOCAB, size=(B, S)).astype(np.int64)
    lengths = rng.integers(1, steps + 1, size=(B,)).astype(np.int64)
    lengths[0] = steps
    s_e, s_h = 1 / np.sqrt(EMBED), 1 / np.sqrt(HIDDEN)
    ins = dict(
        x=x, lengths=lengths,
        emb=rng.normal(size=(VOCAB, EMBED)).astype(np.float32),
        W_i=rng.uniform(-s_e, s_e, (HIDDEN, EMBED)).astype(np.float32),
        b_i=rng.uniform(-s_e, s_e, (HIDDEN,)).astype(np.float32),
        W_f=rng.uniform(-s_h, s_h, (HIDDEN, HIDDEN)).astype(np.float32),
        b_f=rng.uniform(-s_h, s_h, (HIDDEN,)).astype(np.float32),
        W_h=rng.uniform(-s_h, s_h, (HIDDEN, HIDDEN)).astype(np.float32),
        b_h=rng.uniform(-s_h, s_h, (HIDDEN,)).astype(np.float32),
        W_o=rng.uniform(-s_h, s_h, (HIDDEN, HIDDEN)).astype(np.float32),
        b_o=rng.uniform(-s_h, s_h, (HIDDEN,)).astype(np.float32),
        W_lin=rng.uniform(-s_h, s_h, (NCLS, HIDDEN)).astype(np.float32),
        b_lin=np.zeros((NCLS,), np.float32),
    )

    def npref(steps):
        e = ins["emb"][x]  # [B, S, E]
        h = np.zeros((B, HIDDEN), np.float32)
        outs = np.zeros((steps, B, HIDDEN), np.float32)
        for t in range(steps):
            et_ = e[:, t, :]
            inp = np.maximum(et_ @ ins["W_i"].T + ins["b_i"], 0)
            hf = 1 / (1 + np.exp(-(h @ ins["W_f"].T + ins["b_f"])))
            hh = np.tanh(h @ ins["W_h"].T + ins["b_h"])
            h = hf + hh * inp
            outs[t] = h
        li = outs[np.minimum(lengths, steps) - 1, np.arange(B)]
        lin = li @ ins["W_o"].T + ins["b_o"]
        lg = lin @ ins["W_lin"].T + ins["b_lin"]
        lg = lg - lg.max(1, keepdims=True)
        return lg - np.log(np.exp(lg).sum(1, keepdims=True))

    expected = npref(steps)
    res = _run(ins, steps=steps, trace=False)
    got = res.results[0]["out"]
    err = np.linalg.norm(got - expected) / np.linalg.norm(expected)
    print("expected[:3]:", expected[:3])
    print("got[:3]:", got[:3])
    print("rel_err:", err)


# revision 15
# speedup vs baseline: 2.1732x; 2.1732x over previous
"""Trainium2 Bass kernel for nn_FCLSTM: embedding -> custom LSTM-ish recurrence -> select -> linear -> log_softmax.

Batch-sharded: each of 8 cores owns 8 batch rows. Per core:
  phase B: gather embeddings for its 4096 tokens, inp = relu(e @ Wi.T + bi) -> DRAM
  phase C: 512-step recurrence on its 8 rows (full H=1024), h stored per step to ring
  phase D: select h at lengths-1, tiny AllGather [8,1024]->[64,1024], replicated final linear+log_softmax

Self-contained: hardcodes shapes. kernel(**inputs) takes full numpy inputs, returns [64, 2] fp32.
"""
import os
import numpy as np

import concourse.bacc as bacc
import concourse.bass as bass
import concourse.mybir as mybir
from concourse import library_config  # noqa: F401
from concourse.tile import TileContext
from concourse.masks import make_identity
from concourse.bass_utils import run_bass_kernel_spmd

VOCAB, EMBED, HIDDEN, NCLS = 32000, 512, 1024, 2
B, S = 64, 512
NCORES = 8
BC = B // NCORES               # 8 batch rows per core
NEC = EMBED // 128             # 4 embed contraction chunks
NKC = HIDDEN // 128            # 8 hidden contraction chunks
F16 = mybir.dt.float16
F32 = mybir.dt.float32
I32 = mybir.dt.int32
AF = mybir.ActivationFunctionType

_CACHE = {}


def _build(steps=S):
    nc = bacc.Bacc("TRN2", target_bir_lowering=False, debug=False, num_devices=NCORES)
    ntok = BC * steps
    ntt = ntok // 128              # token tiles (32 for full S)

    # ---------- inputs ----------
    emb = nc.dram_tensor("emb", [VOCAB, EMBED], F16, kind="ExternalInput")
    wi = nc.dram_tensor("wi", [EMBED, HIDDEN], F16, kind="ExternalInput")
    bi = nc.dram_tensor("bi", [1, HIDDEN], F16, kind="ExternalInput")
    wf = nc.dram_tensor("wf", [HIDDEN, HIDDEN], F16, kind="ExternalInput")
    wh = nc.dram_tensor("wh", [HIDDEN, HIDDEN], F16, kind="ExternalInput")
    bf_r = nc.dram_tensor("bf_r", [1, HIDDEN], F16, kind="ExternalInput")
    bh_r = nc.dram_tensor("bh_r", [1, HIDDEN], F16, kind="ExternalInput")
    wo = nc.dram_tensor("wo", [HIDDEN, HIDDEN], F16, kind="ExternalInput")
    bo_r = nc.dram_tensor("bo_r", [1, HIDDEN], F16, kind="ExternalInput")
    wlin = nc.dram_tensor("wlin", [HIDDEN, NCLS], F16, kind="ExternalInput")
    idxg = nc.dram_tensor("idxg", [128, ntt], I32, kind="ExternalInput")
    selidx = nc.dram_tensor("selidx", [128, 1], I32, kind="ExternalInput")
    out_ext = nc.dram_tensor("out", [B, NCLS], F32, kind="ExternalOutput")

    with TileContext(nc) as tc:
        with (
            tc.tile_pool(name="dram", bufs=1, space="DRAM") as dram,
            tc.tile_pool(name="const", bufs=1) as cst,
            tc.tile_pool(name="w", bufs=1) as wpool,
            tc.tile_pool(name="ph", bufs=3) as ph,
            tc.tile_pool(name="ppsum", bufs=2, space="PSUM") as ppsum,
            tc.tile_pool(name="gpsum", bufs=2, space="PSUM") as gpsum,
            tc.tile_pool(name="tpsum", bufs=2, space="PSUM") as tpsum,
            tc.tile_pool(name="rec", bufs=3) as rec,
            tc.tile_pool(name="inp", bufs=6) as inpp,
        ):
            # ---------- DRAM scratch ----------
            inp_dram = dram.tile([ntok, HIDDEN], F16)
            ring = dram.tile([ntok, HIDDEN], F16)
            agin = dram.tile([BC, HIDDEN], F16)
            gath = dram.tile([B, HIDDEN], F16, addr_space="Shared")

            # ---------- constants / weights to SBUF ----------
            ones = cst.tile([1, 128], F16, tag="ones")
            nc.vector.memset(ones[:], 1.0)
            ident128 = cst.tile([128, 128], F16, tag="id128")
            make_identity(nc, ident128[:])
            ident8 = cst.tile([8, 8], F16, tag="id8")
            make_identity(nc, ident8[:])
            ident8m = cst.tile([40, 8], F16, tag="id8m")
            nc.vector.tensor_copy(out=ident8m[32:40, :], in_=ident8[:])
            ident64 = cst.tile([64, 64], F16, tag="id64")
            make_identity(nc, ident64[:])

            wi_sb = cst.tile([128, NEC * HIDDEN], F16, tag="wi")
            for e in range(NEC):
                nc.sync.dma_start(out=wi_sb[:, e * HIDDEN:(e + 1) * HIDDEN],
                                  in_=wi[e * 128:(e + 1) * 128, :])
            bi_sb = cst.tile([1, HIDDEN], F16, tag="bi")
            nc.sync.dma_start(out=bi_sb[:], in_=bi[:])
            bf_sb = cst.tile([1, HIDDEN], F16, tag="bf")
            nc.sync.dma_start(out=bf_sb[:], in_=bf_r[:])
            bh_sb = cst.tile([1, HIDDEN], F16, tag="bh")
            nc.sync.dma_start(out=bh_sb[:], in_=bh_r[:])
            bo_sb = cst.tile([1, HIDDEN], F16, tag="bo")
            nc.sync.dma_start(out=bo_sb[:], in_=bo_r[:])

            wf_sb = wpool.tile([128, NKC * HIDDEN], F16, tag="wf")
            wh_sb = wpool.tile([128, NKC * HIDDEN], F16, tag="wh")
            for k in range(NKC):
                nc.sync.dma_start(out=wf_sb[:, k * HIDDEN:(k + 1) * HIDDEN],
                                  in_=wf[k * 128:(k + 1) * 128, :])
                nc.sync.dma_start(out=wh_sb[:, k * HIDDEN:(k + 1) * HIDDEN],
                                  in_=wh[k * 128:(k + 1) * 128, :])

            # ---------- phase B: inp = relu(e @ Wi.T + bi) for this core's tokens ----------
            idx_sb = cst.tile([128, ntt], I32, tag="idx")
            nc.sync.dma_start(out=idx_sb[:], in_=idxg[:, :])
            for i in range(ntt):
                gt = ph.tile([128, EMBED], F16, tag="gt")
                nc.gpsimd.indirect_dma_start(
                    out=gt[:], out_offset=None,
                    in_=emb[:, :],
                    in_offset=bass.IndirectOffsetOnAxis(ap=idx_sb[:, i:i + 1], axis=0))
                pte = tpsum.tile([128, 512], F16, tag="pt")
                for e in range(NEC):
                    nc.tensor.transpose(out=pte[:, e * 128:(e + 1) * 128],
                                        in_=gt[:, e * 128:(e + 1) * 128],
                                        identity=ident128[:])
                eT = ph.tile([128, EMBED], F16, tag="eT")
                nc.vector.tensor_copy(out=eT[:], in_=pte[:])
                inp_sb = ph.tile([128, HIDDEN], F16, tag="inp_sb")
                for n in range(2):
                    ns = slice(n * 512, (n + 1) * 512)
                    pu = ppsum.tile([128, 512], F32, tag="pu")
                    for e in range(NEC):
                        nc.tensor.matmul(out=pu[:], lhsT=eT[:, e * 128:(e + 1) * 128],
                                         rhs=wi_sb[:, e * HIDDEN + n * 512:e * HIDDEN + (n + 1) * 512],
                                         start=(e == 0), stop=False)
                    nc.tensor.matmul(out=pu[:], lhsT=ones[:, 0:128], rhs=bi_sb[:, ns],
                                     start=False, stop=True)
                    nc.scalar.activation(inp_sb[:, ns], pu[:], AF.Relu)
                nc.sync.dma_start(out=inp_dram[i * 128:(i + 1) * 128, :], in_=inp_sb[:])

            # ---------- phase C: recurrence over this core's 8 rows ----------
            # hT chunks: hT[:, 8k:8k+8] = h[:, 128k:128(k+1)].T  ([128, 8] per chunk)
            # 4-way col-tiled PSUM quarters (pg is [128, 512]):
            #   zf-n0 @ rows 0:8, zh-n0 @ 32:40, zf-n1 @ 64:72, zh-n1 @ 96:104
            # tanh(z) = 2*sigmoid(2z) - 1, so ONE sigmoid act covers all quarters
            # via a per-partition scale vector (1 on zf rows, 2 on zh rows).
            scl = cst.tile([128, 1], F32, tag="scl")
            nc.vector.memset(scl[:], 1.0)
            nc.vector.memset(scl[32:40, :], 2.0)
            nc.vector.memset(scl[96:104, :], 2.0)

            # State tiles live at two partition bases so TensorTensor ops stay
            # base-aligned: n0 data (H cols 0:512) at rows 0:8, n1 data
            # (H cols 512:1024) at rows 64:72.
            hT0 = rec.tile([128, NKC * BC], F16, tag="hT")
            nc.vector.memset(hT0[:], 0.0)
            hnw_prev = None
            KORD = [4, 5, 0, 1, 6, 7, 2, 3]  # consumption order matches chunk readiness

            inp_tiles = {}

            def load_inp(t):
                if t >= steps:
                    return
                it = inpp.tile([40, 512], F16, tag="inp")
                nc.sync.dma_start(out=it[0:BC, :], in_=inp_dram[t * BC:(t + 1) * BC, 0:512])
                nc.sync.dma_start(out=it[32:32 + BC, :], in_=inp_dram[t * BC:(t + 1) * BC, 512:1024])
                inp_tiles[t] = it

            load_inp(0)
            load_inp(1)

            for t in range(steps):
                load_inp(t + 2)
                inp = inp_tiles.pop(t)

                pgA = gpsum.tile([128, 256], F32, tag="pgA", name="pgA")
                pgB = gpsum.tile([128, 256], F32, tag="pgB", name="pgB")
                pgs = [pgA, pgB]
                if t > 0:
                    hTc = rec.tile([128, NKC * BC], F16, tag="hT")
                    pt = tpsum.tile([128, 512], F16, tag="pt")

                    def tr2(k0):
                        # transpose prev-step chunks k0, k0+1 and land them in hTc
                        for kk in (k0, k0 + 1):
                            if kk < 4:
                                src = hnw_prev[0:BC, kk * 128:(kk + 1) * 128]
                                idn = ident8[:]
                            else:
                                src = hnw_prev[32:32 + BC, (kk - 4) * 128:(kk - 3) * 128]
                                idn = ident8m[32:40, :]
                            nc.tensor.transpose(out=pt[:, kk * BC:(kk + 1) * BC],
                                                in_=src, identity=idn)
                        # early chunks copied by Scalar; late ones by DVE
                        # (GpSimd cannot access PSUM)
                        ceng = {4: nc.scalar, 0: nc.scalar, 6: nc.vector, 2: nc.vector}[k0]
                        if ceng is nc.scalar:
                            ceng.copy(out=hTc[:, k0 * BC:(k0 + 2) * BC],
                                      in_=pt[:, k0 * BC:(k0 + 2) * BC])
                        else:
                            ceng.tensor_copy(out=hTc[:, k0 * BC:(k0 + 2) * BC],
                                             in_=pt[:, k0 * BC:(k0 + 2) * BC])
                else:
                    hTc = hT0

                    def tr2(k0):
                        pass

                def gates4(k, p, stop):
                    # 4 concurrent MMs: (f,n0) (h,n0) (f,n1) (h,n1) on col groups 0..3
                    lhs = hTc[:, k * BC:(k + 1) * BC]
                    for q in range(4):
                        nn, wsb = q % 2, (wf_sb if q < 2 else wh_sb)
                        w0 = k * HIDDEN + nn * 512 + p * 256
                        nc.tensor.matmul(out=pgs[p][32 * q:32 * q + BC, :], lhsT=lhs,
                                         rhs=wsb[:, w0:w0 + 256],
                                         start=False, stop=stop, tile_position=(0, 32 * q))

                def bias4(p):
                    for q in range(4):
                        nn, bsb = q % 2, (bf_sb if q < 2 else bh_sb)
                        b0 = nn * 512 + p * 256
                        nc.tensor.matmul(out=pgs[p][32 * q:32 * q + BC, :], lhsT=ones[:, 0:BC],
                                         rhs=bsb[:, b0:b0 + 256],
                                         start=True, stop=False, tile_position=(0, 32 * q))

                sg = rec.tile([40, 512], F16, tag="sg")
                th = rec.tile([40, 512], F16, tag="th")
                hnw = rec.tile([40, 512], F16, tag="hnw")

                def tail(p):
                    # two acts per piece, each spanning both n-halves (junk rows
                    # 8:32 computed for free); n1 chain on DVE, n0 on GpSimd
                    pc = slice(p * 256, (p + 1) * 256)
                    nc.scalar.activation(th[0:40, pc], pgs[p][64:104, :], AF.Tanh)
                    nc.scalar.activation(sg[0:40, pc], pgs[p][0:40, :], AF.Sigmoid)
                    for nn, eng in ((1, nc.vector), (0, nc.gpsimd)):
                        b = 32 * nn
                        eng.tensor_mul(out=hnw[b:b + BC, pc], in0=th[b:b + BC, pc],
                                       in1=inp[b:b + BC, pc])
                        eng.tensor_add(out=hnw[b:b + BC, pc], in0=hnw[b:b + BC, pc],
                                       in1=sg[b:b + BC, pc])

                # PE program order: prev-step transposes interleave as fillers
                tr2(4)
                bias4(0)
                tr2(0)
                gates4(4, 0, False)
                gates4(5, 0, False)
                gates4(0, 0, False)
                gates4(1, 0, False)
                tr2(6)
                gates4(6, 0, False)
                gates4(7, 0, False)
                tr2(2)
                gates4(2, 0, False)
                gates4(3, 0, True)
                tail(0)
                bias4(1)
                for i, k in enumerate(KORD):
                    gates4(k, 1, i == 7)
                tail(1)

                nc.sync.dma_start(out=ring[t * BC:(t + 1) * BC, 0:512], in_=hnw[0:BC, :])
                nc.sync.dma_start(out=ring[t * BC:(t + 1) * BC, 512:1024],
                                  in_=hnw[32:32 + BC, :])
                hnw_prev = hnw

            # ---------- phase D: select + AllGather + linear + log_softmax ----------
            six = cst.tile([128, 1], I32, tag="six")
            nc.sync.dma_start(out=six[:], in_=selidx[:])
            hsel = cst.tile([128, HIDDEN], F16, tag="hsel")
            nc.gpsimd.indirect_dma_start(
                out=hsel[:], out_offset=None,
                in_=ring[:, :],
                in_offset=bass.IndirectOffsetOnAxis(ap=six[:, :1], axis=0))
            nc.sync.dma_start(out=agin[:, :], in_=hsel[0:BC, :])
            nc.gpsimd.collective_compute(
                "AllGather", mybir.AluOpType.bypass,
                replica_groups=[list(range(NCORES))],
                ins=[agin.opt()], outs=[gath.opt()])
            h64 = cst.tile([64, HIDDEN], F16, tag="h64")
            nc.sync.dma_start(out=h64[:], in_=gath[:, :])

            pt2 = tpsum.tile([128, 512], F16, tag="pt")
            for k in range(NKC):
                nc.tensor.transpose(out=pt2[:, k * 64:(k + 1) * 64],
                                    in_=h64[:, k * 128:(k + 1) * 128],
                                    identity=ident64[:])
            hT64 = cst.tile([128, NKC * 64], F16, tag="hT64")
            nc.vector.tensor_copy(out=hT64[:], in_=pt2[:])

            wo_sb = wpool.tile([128, NKC * HIDDEN], F16, tag="wo")
            for k in range(NKC):
                nc.sync.dma_start(out=wo_sb[:, k * HIDDEN:(k + 1) * HIDDEN],
                                  in_=wo[k * 128:(k + 1) * 128, :])
            lin = cst.tile([64, HIDDEN], F16, tag="lin")
            for n in range(2):
                ns = slice(n * 512, (n + 1) * 512)
                pl = ppsum.tile([128, 512], F32, tag="pu")
                for k in range(NKC):
                    nc.tensor.matmul(out=pl[0:64, :], lhsT=hT64[:, k * 64:(k + 1) * 64],
                                     rhs=wo_sb[:, k * HIDDEN + n * 512:k * HIDDEN + (n + 1) * 512],
                                     start=(k == 0), stop=False)
                nc.tensor.matmul(out=pl[0:64, :], lhsT=ones[:, 0:64], rhs=bo_sb[:, ns],
                                 start=False, stop=True)
                nc.vector.tensor_copy(out=lin[:, ns], in_=pl[0:64, :])
            pt3 = tpsum.tile([128, 512], F16, tag="pt")
            for k in range(NKC):
                nc.tensor.transpose(out=pt3[:, k * 64:(k + 1) * 64],
                                    in_=lin[:, k * 128:(k + 1) * 128],
                                    identity=ident64[:])
            linT = cst.tile([128, NKC * 64], F16, tag="linT")
            nc.vector.tensor_copy(out=linT[:], in_=pt3[:])
            wl_sb = cst.tile([128, NKC * NCLS], F16, tag="wl")
            for k in range(NKC):
                nc.sync.dma_start(out=wl_sb[:, k * NCLS:(k + 1) * NCLS],
                                  in_=wlin[k * 128:(k + 1) * 128, :])
            pz = ppsum.tile([128, 512], F32, tag="pu")
            for k in range(NKC):
                nc.tensor.matmul(out=pz[0:64, 0:NCLS], lhsT=linT[:, k * 64:(k + 1) * 64],
                                 rhs=wl_sb[:, k * NCLS:(k + 1) * NCLS],
                                 start=(k == 0), stop=(k == NKC - 1))
            m = cst.tile([64, 1], F32, tag="m")
            nc.vector.tensor_reduce(out=m[:], in_=pz[0:64, 0:NCLS], axis=mybir.AxisListType.X,
                                    op=mybir.AluOpType.max)
            xm = cst.tile([64, NCLS], F32, tag="xm")
            nc.vector.tensor_scalar(out=xm[:], in0=pz[0:64, 0:NCLS], scalar1=m[:], scalar2=None,
                                    op0=mybir.AluOpType.subtract)
            esum = cst.tile([64, 1], F32, tag="esum")
            ex = cst.tile([64, NCLS], F32, tag="ex")
            nc.scalar.activation(ex[:], xm[:], AF.Exp, accum_out=esum[:])
            lns = cst.tile([64, 1], F32, tag="lns")
            nc.scalar.activation(lns[:], esum[:], AF.Ln)
            res = cst.tile([64, NCLS], F32, tag="res")
            nc.vector.tensor_scalar(out=res[:], in0=xm[:], scalar1=lns[:], scalar2=None,
                                    op0=mybir.AluOpType.subtract)
            nc.sync.dma_start(out=out_ext[:, :], in_=res[:])

    nc.compile()
    return nc


def _prep(x, lengths, emb, W_i, b_i, W_f, b_f, W_h, b_h, W_o, b_o, W_lin, b_lin,
          steps=S):
    f16 = np.float16
    emb16 = np.ascontiguousarray(emb.astype(f16))
    wiT = np.ascontiguousarray(W_i.T.astype(f16))
    wfT = np.ascontiguousarray(W_f.T.astype(f16))
    whT = np.ascontiguousarray(W_h.T.astype(f16))
    woT = np.ascontiguousarray(W_o.T.astype(f16))
    wlT = np.ascontiguousarray(W_lin.T.astype(f16))
    maps = []
    for c in range(NCORES):
        rows = slice(c * BC, (c + 1) * BC)
        tok = np.ascontiguousarray(x[rows, :steps].T).reshape(-1)  # t-major [steps*BC]
        ntt = (steps * BC) // 128
        idxg = np.ascontiguousarray(tok.reshape(ntt, 128).T).astype(np.int32)
        ln = np.minimum(lengths[rows].astype(np.int64), steps)
        sel = ((ln - 1) * BC + np.arange(BC)).astype(np.int32)
        selpad = np.zeros((128, 1), np.int32)
        selpad[:BC, 0] = sel
        maps.append({
            "emb": emb16,
            "wi": wiT,
            "bi": b_i[None, :].astype(f16),
            "wf": wfT,
            "wh": whT,
            "bf_r": b_f[None, :].astype(f16),
            "bh_r": b_h[None, :].astype(f16),
            "wo": woT,
            "bo_r": b_o[None, :].astype(f16),
            "wlin": wlT,
            "idxg": idxg,
            "selidx": selpad,
        })
    return maps


def _run(inputs, steps=S, trace=False):
    key = steps
    if key not in _CACHE:
        _CACHE[key] = _build(steps)
    nc = _CACHE[key]
    maps = _prep(**inputs, steps=steps)
    res = run_bass_kernel_spmd(nc, maps, core_ids=list(range(NCORES)), trace=trace)
    return res


def kernel(**inputs) -> np.ndarray:
    res = _run(inputs, steps=S, trace=False)
    return res.results[0]["out"]


if __name__ == "__main__":
    steps = int(os.environ.get("KSTEPS", "16"))
    rng = np.random.default_rng(0)
    x = rng.integers(0, VOCAB, size=(B, S)).astype(np.int64)
    lengths = rng.integers(1, steps + 1, size=(B,)).astype(np.int64)
    lengths[0] = steps
    s_e, s_h = 1 / np.sqrt(EMBED), 1 / np.sqrt(HIDDEN)
    ins = dict(
        x=x, lengths=lengths,
        emb=rng.normal(size=(VOCAB, EMBED)).astype(np.float32),
        W_i=rng.uniform(-s_e, s_e, (HIDDEN, EMBED)).astype(np.float32),
        b_i=rng.uniform(-s_e, s_e, (HIDDEN,)).astype(np.float32),
        W_f=rng.uniform(-s_h, s_h, (HIDDEN, HIDDEN)).astype(np.float32),
        b_f=rng.uniform(-s_h, s_h, (HIDDEN,)).astype(np.float32),
        W_h=rng.uniform(-s_h, s_h, (HIDDEN, HIDDEN)).astype(np.float32),
        b_h=rng.uniform(-s_h, s_h, (HIDDEN,)).astype(np.float32),
        W_o=rng.uniform(-s_h, s_h, (HIDDEN, HIDDEN)).astype(np.float32),
        b_o=rng.uniform(-s_h, s_h, (HIDDEN,)).astype(np.float32),
        W_lin=rng.uniform(-s_h, s_h, (NCLS, HIDDEN)).astype(np.float32),
        b_lin=np.zeros((NCLS,), np.float32),
    )

    def npref(steps):
        e = ins["emb"][x]  # [B, S, E]
        h = np.zeros((B, HIDDEN), np.float32)
        outs = np.zeros((steps, B, HIDDEN), np.float32)
        for t in range(steps):
            et_ = e[:, t, :]
            inp = np.maximum(et_ @ ins["W_i"].T + ins["b_i"], 0)
            hf = 1 / (1 + np.exp(-(h @ ins["W_f"].T + ins["b_f"])))
            hh = np.tanh(h @ ins["W_h"].T + ins["b_h"])
            h = hf + hh * inp
            outs[t] = h
        li = outs[np.minimum(lengths, steps) - 1, np.arange(B)]
        lin = li @ ins["W_o"].T + ins["b_o"]
        lg = lin @ ins["W_lin"].T + ins["b_lin"]
        lg = lg - lg.max(1, keepdims=True)
        return lg - np.log(np.exp(lg).sum(1, keepdims=True))

    expected = npref(steps)
    res = _run(ins, steps=steps, trace=False)
    got = res.results[0]["out"]
    err = np.linalg.norm(got - expected) / np.linalg.norm(expected)
    print("expected[:3]:", expected[:3])
    print("got[:3]:", got[:3])
    print("rel_err:", err)
